# revision 24
# baseline (speedup 1.0000x reference)
"""Grouped-experts SwiGLU MoE kernel for Trainium2 (8 NeuronCores).

Problem: x [8192, 2048] f32, 8 experts with w1/w3 [8, 1408, 2048] and
w2 [8, 2048, 1408]; tokens are expert-contiguous with a per-expert count
vector. out[t] = (silu(x_t @ w1_e.T) * (x_t @ w3_e.T)) @ w2_e.T for the
expert e owning token t.

Sharding: pure expert parallelism. Core e receives expert e's 1024-token
tile (dynamic-slice semantics of the reference) plus expert e's weights,
and computes the full SwiGLU MLP for that tile. No collectives.

Performance structure (PE-bound problem: 1056 matmuls x 512 free-dim
~= 225 us/core at 2.4 GHz; ~305 us measured on shared axon trn2, which
runs the PE near the documented P0 2.0 GHz state under sustained load):
  - all streamed operands are fp16 (quantized host-side, rel err
    ~5e-4 vs the 2e-2 gate): same 1 cycle/row PE rate as f32r but half
    the HBM traffic, so DMA hides completely under PE work.
  - stage-1 accumulation is k-streamed: the contraction k-loop is
    outermost per PSUM group and the x k-slices follow the first
    weight chunks in DMA issue order, so the first matmul starts ~2 us
    into the kernel instead of waiting for all of x.
  - weights stream on the ACT HWDGE queue, x/out on the SP queue: a
    buffer-rotation wait on one stream can't head-of-line-block the
    other.
  - contraction dims (D for stage 1, H for stage 2) live on SBUF
    partitions; all tensors are packed [p, ktile, free] in DRAM so
    every DMA is a contiguous partition-row load and the matmuls need
    no on-device transposes.
  - one rotating 8-bank PSUM tag serves both stages; w1/w3 prefetch
    4 tiles deep, w2 tiles are all resident (fp16).
  - a post-compile BIR pass (_dedup_ldweights) drops the redundant
    per-matmul LDWEIGHTS that Tile emits when consecutive matmuls
    share a stationary operand (1067 -> 440 loads, ~2% measured).

Stage 1 computes hT [H, T] = silu(w1 xT) * (w3 xT) per 128-row h-tile
(PSUM [128h, 512t] x2 token blocks, contraction over 16 D-tiles);
stage 2 computes out [T, D] tt-outer/k-mid/db-inner so one stationary
hT token-tile load serves all four dim-blocks (PSUM [128t, 512d],
contraction over 11 H-tiles).

_build_nc(reps=R, hw_loop=True) wraps the body in a device-side
tc.For_i loop for the timing protocol in test.py (constant NEFF size).
"""

from contextlib import ExitStack

import numpy as np

import concourse.bass as bass
import concourse.mybir as mybir
import concourse.tile as tile
from concourse import bacc
from concourse.bass import ts
from concourse.bass_utils import run_bass_kernel_spmd

F32 = mybir.dt.float32
F16 = mybir.dt.float16

N_TOKENS = 8192
DIM = 2048
HIDDEN = 1408
N_EXPERTS = 8
CAP = N_TOKENS // N_EXPERTS  # 1024 tokens per core
P = 128
KD = DIM // P  # 16 contraction tiles, stage 1
KH = HIDDEN // P  # 11 contraction tiles, stage 2
TB = 512  # token-block (stage-1 moving free dim)
DB = 512  # dim-block (stage-2 moving free dim)
N_TB = CAP // TB  # 2
N_DB = DIM // DB  # 4
N_TT = CAP // P  # 8 token tiles (stage-2 stationary)
W_PREFETCH = 4  # w1/w3 tile pairs in flight

_CACHED_NC = None


def _build_nc(reps=1, hw_loop=False):
    nc = bacc.Bacc("TRN2", debug=False)
    xQ = nc.dram_tensor("xQ", [P, KD, CAP], F16, kind="ExternalInput").ap()
    w1Q = nc.dram_tensor("w1Q", [KH, P, KD, P], F16, kind="ExternalInput").ap()
    w3Q = nc.dram_tensor("w3Q", [KH, P, KD, P], F16, kind="ExternalInput").ap()
    w2Q = nc.dram_tensor("w2Q", [N_DB, P, KH, DB], F16, kind="ExternalInput").ap()
    out = nc.dram_tensor("out", [CAP, DIM], F32, kind="ExternalOutput").ap()

    with tile.TileContext(nc) as tc, ExitStack() as ctx:
        xpool = ctx.enter_context(tc.tile_pool(name="xpool", bufs=1))
        hpool = ctx.enter_context(tc.tile_pool(name="hpool", bufs=1))
        wpool = ctx.enter_context(tc.tile_pool(name="wpool", bufs=W_PREFETCH))
        w2pool = ctx.enter_context(tc.tile_pool(name="w2pool", bufs=N_DB))
        tmppool = ctx.enter_context(tc.tile_pool(name="tmppool", bufs=3))
        opool = ctx.enter_context(tc.tile_pool(name="opool", bufs=4))
        pspool = ctx.enter_context(tc.tile_pool(name="pspool", bufs=8, space="PSUM"))

        def one_rep():
            # x, transposed: [d-inner(part), d-tile, t] — resident all of
            # stage 1. Loaded per d-tile slice, interleaved below with the
            # first weight tiles so compute starts as soon as slice 0 lands.
            x_sb = xpool.tile([P, KD, CAP], F16)
            # hT tiles: [h-inner(part), h-tile, t]
            h_sb = hpool.tile([P, KH, CAP], F16)

            w1_t, w3_t = [], []

            def load_pair(ht, chunks=1):
                w1_sb = wpool.tile([P, KD, P], F16, tag="w1")
                w3_sb = wpool.tile([P, KD, P], F16, tag="w3")
                kc = KD // chunks
                for c in range(chunks):
                    nc.scalar.dma_start(
                        w1_sb[:, ts(c, kc)], w1Q[ht, :, ts(c, kc)]
                    )
                    nc.scalar.dma_start(
                        w3_sb[:, ts(c, kc)], w3Q[ht, :, ts(c, kc)]
                    )
                w1_t.append(w1_sb)
                w3_t.append(w3_sb)

            # Queue split: weights stream on the ACT HWDGE queue, x slices
            # and output tiles on the SP queue — a buffer-rotation wait on
            # one stream can never head-of-line-block the other. The first
            # pair is k-chunked so matmul k=0 starts ~1 us in.
            w1_sb0 = wpool.tile([P, KD, P], F16, tag="w1")
            w3_sb0 = wpool.tile([P, KD, P], F16, tag="w3")
            w1_t.append(w1_sb0)
            w3_t.append(w3_sb0)
            ck = [(0, 2), (2, 2), (4, 4), (8, 8)]  # (start, len) k-chunks
            for c0, cl in ck:
                nc.scalar.dma_start(
                    w1_sb0[:, c0 : c0 + cl], w1Q[0, :, c0 : c0 + cl]
                )
                nc.scalar.dma_start(
                    w3_sb0[:, c0 : c0 + cl], w3Q[0, :, c0 : c0 + cl]
                )
            for ko in range(KD):
                nc.sync.dma_start(x_sb[:, ko], xQ[:, ko])
            for ht in range(1, W_PREFETCH):
                load_pair(ht)

            # Stage 1: per h-tile, k-streamed accumulation over 4 PSUM banks
            # (ps1/ps3 x 2 token blocks); banks rotate 8-wide across tiles.
            for ht in range(KH):
                if ht + W_PREFETCH < KH:
                    load_pair(ht + W_PREFETCH)
                if ht == 0:
                    w2_t = []
                if ht in (2, 4, 6, 8):
                    # w2 tiles for stage 2, off the critical path (ACT queue)
                    db = (ht - 2) // 2
                    w2_sb = w2pool.tile([P, KH, DB], F16, tag="w2")
                    nc.scalar.dma_start(w2_sb[:], w2Q[db])
                    w2_t.append(w2_sb)
                w1_sb, w3_sb = w1_t[ht], w3_t[ht]
                ps1 = [
                    pspool.tile([P, TB], F32, tag="ps", name=f"ps1_{ht}_{tb}")
                    for tb in range(N_TB)
                ]
                ps3 = [
                    pspool.tile([P, TB], F32, tag="ps", name=f"ps3_{ht}_{tb}")
                    for tb in range(N_TB)
                ]
                for k in range(KD):
                    st, sp = (k == 0), (k == KD - 1)
                    for tb in range(N_TB):
                        nc.tensor.matmul(
                            ps1[tb][:], w1_sb[:, k],
                            x_sb[:, k, ts(tb, TB)], start=st, stop=sp,
                        )
                    for tb in range(N_TB):
                        nc.tensor.matmul(
                            ps3[tb][:], w3_sb[:, k],
                            x_sb[:, k, ts(tb, TB)], start=st, stop=sp,
                        )
                for tb in range(N_TB):
                    sil = tmppool.tile([P, TB], F32, tag="sil")
                    nc.scalar.activation(
                        sil[:], ps1[tb][:], mybir.ActivationFunctionType.Silu
                    )
                    nc.vector.tensor_mul(
                        h_sb[:, ht, ts(tb, TB)], sil[:], ps3[tb][:]
                    )

            # Stage 2: out = hT.T @ w2.T — stationary hT token-tiles,
            # moving w2 dim-blocks. tt-outer/k-mid/db-inner: one stationary
            # h-tile load serves all four dim-blocks (4 matmuls per
            # LDWEIGHTS after dedup), with 4 PSUM groups open per tt.
            for tt in range(N_TT):
                ps2 = [
                    pspool.tile([P, DB], F32, tag="ps", name=f"ps2_{tt}_{db}")
                    for db in range(N_DB)
                ]
                for k in range(KH):
                    st, sp = (k == 0), (k == KH - 1)
                    for db in range(N_DB):
                        nc.tensor.matmul(
                            ps2[db][:], h_sb[:, k, ts(tt, P)], w2_t[db][:, k],
                            start=st, stop=sp,
                        )
                for db in range(N_DB):
                    ot = opool.tile([P, DB], F32, tag="ot", name=f"ot_{db}")
                    nc.vector.tensor_copy(ot[:], ps2[db][:])
                    nc.sync.dma_start(out[ts(tt, P), ts(db, DB)], ot[:])

        if hw_loop and reps > 1:
            # constant-size NEFF: body emitted once, looped on-device
            # (used for timing; one all-engine barrier per iteration)
            with tc.For_i(0, reps):
                one_rep()
        else:
            for _ in range(reps):
                one_rep()

    nc.compile()
    _dedup_ldweights(nc)
    return nc


def _dedup_ldweights(nc):
    """Drop back-to-back redundant LDWEIGHTS in the tile-lowered BIR.

    Tile's lowering emits one InstLdweights per InstMatmult even when
    consecutive matmuls share the stationary operand (the PE array keeps
    weights across matmuls with ldweights=false, so the reload is pure
    overhead — measured ~67-90 ns serialized per load). Runs after
    bacc.compile() (post move_matmul_waits_to_ldweights). A duplicate is
    removed only when (a) its operand AP is byte-identical to the
    surviving PE weight load with only Matmult(ldweights=false)/
    EventSemaphore in between on the PE stream, (b) it carries no
    semaphore updates, and (c) its waits are a subset of waits already
    observed on the PE stream since that load (sems are monotonic, so
    the condition is already guaranteed in the in-order PE stream).
    """
    import orjson

    j = orjson.loads(mybir.module_to_json_string(nc.m))
    removed = 0
    for fn in j["functions"]:
        for blk in fn["blocks"]:
            insts = blk.get("instructions", [])
            keep = []
            last_sig = None
            last_waits = []
            for inst in insts:
                if inst.get("engine") != "PE":
                    keep.append(inst)
                    continue
                op = inst.get("opcode")
                if op == "Ldweights":
                    sig = orjson.dumps(
                        [
                            inst.get("ins"),
                            inst.get("tile_position"),
                            inst.get("tile_size"),
                            inst.get("perf_mode"),
                            inst.get("is_transpose"),
                        ]
                    )
                    si = inst.get("sync_info") or {}
                    waits = [orjson.dumps(w) for w in (si.get("on_wait") or [])]
                    if (
                        sig == last_sig
                        and not si.get("on_update")
                        and all(w in last_waits for w in waits)
                    ):
                        removed += 1
                        continue
                    last_sig = sig
                    last_waits = waits
                elif op == "Matmult":
                    if inst.get("ldweights"):
                        last_sig = None  # self-loading matmul clobbers
                    else:
                        si = inst.get("sync_info") or {}
                        last_waits += [
                            orjson.dumps(w) for w in (si.get("on_wait") or [])
                        ]
                elif op != "EventSemaphore":
                    last_sig = None  # unknown PE op: be conservative
                keep.append(inst)
            blk["instructions"] = keep
    if removed:
        nc.m = mybir.module_from_json_bytes(orjson.dumps(j))
    return removed


def _get_nc():
    global _CACHED_NC
    if _CACHED_NC is None:
        _CACHED_NC = _build_nc()
    return _CACHED_NC


def _pack_inputs(x, w1, w2, w3, read_starts):
    """Per-core input dicts with DMA-optimal (partition-major) layouts."""
    in_maps = []
    for e in range(N_EXPERTS):
        s = int(read_starts[e])
        xe = x[s : s + CAP]  # [CAP, DIM]
        xQ = np.ascontiguousarray(
            xe.T.reshape(KD, P, CAP).transpose(1, 0, 2)
        ).astype(np.float16)
        w1Q = np.ascontiguousarray(
            w1[e].T.reshape(KD, P, KH, P).transpose(2, 1, 0, 3)
        ).astype(np.float16)
        w3Q = np.ascontiguousarray(
            w3[e].T.reshape(KD, P, KH, P).transpose(2, 1, 0, 3)
        ).astype(np.float16)
        w2Q = np.ascontiguousarray(
            w2[e].T.reshape(KH, P, N_DB, DB).transpose(2, 1, 0, 3)
        ).astype(np.float16)
        in_maps.append({"xQ": xQ, "w1Q": w1Q, "w3Q": w3Q, "w2Q": w2Q})
    return in_maps


def kernel(x, num_tokens_per_expert, w1, w2, w3):
    x = np.ascontiguousarray(np.asarray(x, dtype=np.float32))
    w1 = np.asarray(w1, dtype=np.float32)
    w2 = np.asarray(w2, dtype=np.float32)
    w3 = np.asarray(w3, dtype=np.float32)
    counts = np.asarray(num_tokens_per_expert).astype(np.int64)

    offsets = np.cumsum(counts)
    starts = offsets - counts
    # jax.lax.dynamic_slice clamps the read start so the slice is in-bounds.
    read_starts = np.clip(starts, 0, N_TOKENS - CAP)

    in_maps = _pack_inputs(x, w1, w2, w3, read_starts)
    nc = _get_nc()
    res = run_bass_kernel_spmd(nc, in_maps, core_ids=list(range(N_EXPERTS)))
    ye = [res.results[e]["out"] for e in range(N_EXPERTS)]

    if np.all(counts == CAP):
        # balanced routing: per-expert tiles are disjoint and exactly cover x
        return np.concatenate(ye, axis=0)

    # general case: mask invalid slots, scatter-add to clipped positions
    y = np.zeros((N_TOKENS, DIM), np.float32)
    slot = np.arange(CAP)
    for e in range(N_EXPERTS):
        valid = slot < counts[e]
        pos = np.clip(starts[e] + slot, 0, N_TOKENS - 1)
        np.add.at(y, pos, np.where(valid[:, None], ye[e], 0.0))
    return y
